# revision 9
# baseline (speedup 1.0000x reference)
"""Multi-head attention (B=2,T=2048,D=1024,H=16,DK=64, causal, RoPE) on 8 TRN2 cores.

Sharding: data-parallel over batch (2) x tensor-parallel over heads (16 -> 4 per
core). core = 4*b + g handles batch b, heads [4g..4g+3]. RoPE tables replicated.
Host pre-transposes x and the projection weights, and permutes the q/k head dims
into [x1(32); x2(32)] blocks per head so RoPE is pure elementwise work on chip.
Each core returns a partial output projection; the host sums the 4 head-group
partials per batch and adds the output bias.
"""

import sys

for _p in ("/opt/trn_rl_repo", "/root/.axon_site/_ro/trn_rl_repo"):
    if _p not in sys.path:
        sys.path.append(_p)

import numpy as np

from concourse import bacc, tile, mybir
from concourse.bass2jax import _bass_exec_p, install_neuronx_cc_hook

B, T, D, H, DK = 2, 2048, 1024, 16, 64
G = 4          # heads per core
DSH = G * DK   # 256 sharded head dims per core
NCORES = 8
KT = D // 128  # 8 contraction tiles for projections
NTT = T // 128  # 16 row tiles
NCH = T // 512  # 4 column chunks
F32 = mybir.dt.float32
F32R = mybir.dt.float32r
BF16 = mybir.dt.bfloat16

_CACHE = {}


DEBUG_DUMPS = False


def _build_bass():
    nc = bacc.Bacc("TRN2", target_bir_lowering=False, debug=False)

    xT = nc.dram_tensor("xT", [D, T], F32, kind="ExternalInput").ap()
    wqT = nc.dram_tensor("wqT", [D, DSH], F32, kind="ExternalInput").ap()
    wkT = nc.dram_tensor("wkT", [D, DSH], F32, kind="ExternalInput").ap()
    wvT = nc.dram_tensor("wvT", [D, DSH], F32, kind="ExternalInput").ap()
    woT = nc.dram_tensor("woT", [DSH, D], F32, kind="ExternalInput").ap()
    bqk = nc.dram_tensor("bqk", [128, 4], F32, kind="ExternalInput").ap()
    bv = nc.dram_tensor("bv", [1, DSH], F32, kind="ExternalInput").ap()
    cc = nc.dram_tensor("cc", [128, T], F32, kind="ExternalInput").ap()
    ss = nc.dram_tensor("ss", [128, T], F32, kind="ExternalInput").ap()
    m01 = nc.dram_tensor("m01", [128, 128], BF16, kind="ExternalInput").ap()
    ones = nc.dram_tensor("ones", [1, 128], F32, kind="ExternalInput").ap()
    out = nc.dram_tensor("out", [T, D], F32, kind="ExternalOutput").ap()
    if DEBUG_DUMPS:
        dbg_q = nc.dram_tensor("dbg_q", [128, 2 * T], F32, kind="ExternalOutput").ap()
        dbg_k = nc.dram_tensor("dbg_k", [128, 2 * T], F32, kind="ExternalOutput").ap()
        dbg_v = nc.dram_tensor("dbg_v", [128, G * NTT * 65], F32, kind="ExternalOutput").ap()
        dbg_c = nc.dram_tensor("dbg_c", [128, 2 * T], F32, kind="ExternalOutput").ap()

    def r(ap):  # fp32 storage -> fp32r matmul operand
        return ap.bitcast(F32R)

    with tile.TileContext(nc) as tc:
        with (
            tc.tile_pool(name="const", bufs=1) as const,
            tc.tile_pool(name="persist", bufs=1) as persist,
            tc.tile_pool(name="xt", bufs=3) as xtp,
            tc.tile_pool(name="rope", bufs=2) as ropep,
            tc.tile_pool(name="attn", bufs=2) as attnp,
            tc.tile_pool(name="epi", bufs=2) as epip,
        ):
            # ---- resident tensors ----
            wq_sb = const.tile([128, KT, DSH], F32R)
            wk_sb = const.tile([128, KT, DSH], F32R)
            wv_sb = const.tile([128, KT, DSH], F32R)
            for w_sb, w_dram in ((wq_sb, wqT), (wk_sb, wkT), (wv_sb, wvT)):
                nc.sync.dma_start(out=w_sb, in_=w_dram.rearrange("(k p) n -> p k n", p=128).bitcast(F32R))
            wo_sb = const.tile([128, 2, D], F32R)
            nc.sync.dma_start(out=wo_sb, in_=woT.rearrange("(k p) n -> p k n", p=128).bitcast(F32R))
            cc_sb = const.tile([128, T], F32)
            ss_sb = const.tile([128, T], F32)
            nc.sync.dma_start(out=cc_sb, in_=cc)
            nc.sync.dma_start(out=ss_sb, in_=ss)
            bqk_sb = const.tile([128, 4], F32)
            nc.sync.dma_start(out=bqk_sb, in_=bqk)
            bv_sb = const.tile([1, DSH], F32R)
            nc.sync.dma_start(out=bv_sb, in_=bv.bitcast(F32R))
            m01_sb = const.tile([128, 128], BF16)
            nc.sync.dma_start(out=m01_sb, in_=m01)
            ones_sb = const.tile([1, 128], F32R)
            nc.sync.dma_start(out=ones_sb, in_=ones.bitcast(F32R))

            qT_sb = persist.tile([128, 2, T], F32R)   # [d-tile, t], heads 2*dt+{0,1}
            kT_sb = persist.tile([128, 2, T], F32R)
            v1_sb = persist.tile([128, G, NTT, 65], BF16)  # [s, head, s-tile, d|1]
            nc.vector.memset(v1_sb, 1.0)  # col 64 stays 1.0 (softmax denominators)
            ctxT_sb = persist.tile([128, 2, T], F32R)

            # ---- phase 1: projections + RoPE, one 512-wide t-chunk at a time ----
            with tc.tile_pool(name="ps1", bufs=1, space="PSUM") as ps1:
                for tch in range(NCH):
                    tsl = slice(512 * tch, 512 * tch + 512)
                    qp = [ps1.tile([128, 512], F32, tag=f"qp{dt}", name=f"qp{dt}") for dt in range(2)]
                    kp = [ps1.tile([128, 512], F32, tag=f"kp{dt}", name=f"kp{dt}") for dt in range(2)]
                    vp = [ps1.tile([128, 256], F32, tag=f"vp{i}", name=f"vp{i}") for i in range(4)]
                    for k in range(KT):
                        xt = xtp.tile([128, 512], F32R, tag="xt")
                        nc.sync.dma_start(out=xt, in_=xT[128 * k : 128 * k + 128, tsl].bitcast(F32R))
                        for dt in range(2):
                            dsl = slice(128 * dt, 128 * dt + 128)
                            nc.tensor.matmul(qp[dt], wq_sb[:, k, dsl], xt,
                                             start=(k == 0), stop=(k == KT - 1))
                            nc.tensor.matmul(kp[dt], wk_sb[:, k, dsl], xt,
                                             start=(k == 0), stop=(k == KT - 1))
                        for tt in range(4):
                            nc.tensor.matmul(
                                vp[tt],
                                xt[:, 128 * tt : 128 * tt + 128],
                                wv_sb[:, k, :],
                                start=(k == 0), stop=False)
                    for tt in range(4):  # + bv broadcast along t (rank-1 matmul)
                        nc.tensor.matmul(vp[tt], ones_sb, bv_sb, start=False, stop=True)
                    # v -> [s, d] bf16 slices per head (ones col untouched)
                    for tt in range(4):
                        st = 4 * tch + tt
                        nc.scalar.activation(
                            out=v1_sb[:, :, st, 0:64],
                            in_=vp[tt],
                            func=mybir.ActivationFunctionType.Copy)
                    # q/k: bias add (ACT) then RoPE (vector) into qT_sb/kT_sb
                    for dt in range(2):
                        for which, psumt, dst in ((0, qp[dt], qT_sb), (1, kp[dt], kT_sb)):
                            raw = ropep.tile([128, 512], F32, tag="raw")
                            nc.scalar.activation(
                                out=raw, in_=psumt,
                                func=mybir.ActivationFunctionType.Identity,
                                bias=bqk_sb[:, 2 * which + dt : 2 * which + dt + 1])
                            swp = ropep.tile([128, 512], F32, tag="swp")
                            for blk in range(4):
                                src = blk ^ 1
                                nc.sync.dma_start(
                                    out=swp[32 * blk : 32 * blk + 32, :],
                                    in_=raw[32 * src : 32 * src + 32, :])
                            t1 = ropep.tile([128, 512], F32, tag="t1")
                            t2 = ropep.tile([128, 512], F32, tag="t2")
                            nc.vector.tensor_mul(t1, raw, cc_sb[:, tsl])
                            nc.vector.tensor_mul(t2, swp, ss_sb[:, tsl])
                            nc.vector.tensor_add(dst[:, dt, tsl], t1, t2)

            # ---- phase 2: causal attention, transposed scores ----
            with tc.tile_pool(name="ps2", bufs=1, space="PSUM") as ps2:
                for h in range(G):
                    dt = h // 2
                    ro = 64 * (h % 2)
                    rsl = slice(ro, ro + 64)
                    for j in range(NCH):
                        qsl = slice(512 * j, 512 * j + 512)
                        nst = 4 * j + 4  # s-tiles needed (incl. diagonal)
                        at = attnp.tile([128, NTT, 512], BF16, tag="attnT")
                        for p2 in range(nst // 2):  # scores + exp, 2 s-tiles a time
                            sc = ps2.tile([128, 1024], F32, tag="sc", bufs=2)
                            for i in range(2):
                                st = 2 * p2 + i
                                nc.tensor.matmul(
                                    sc[:, 512 * i : 512 * i + 512],
                                    kT_sb[rsl, dt, 128 * st : 128 * st + 128],
                                    qT_sb[rsl, dt, qsl],
                                    start=True, stop=True)
                            nc.scalar.activation(
                                out=at[:, 2 * p2 : 2 * p2 + 2, :], in_=sc,
                                func=mybir.ActivationFunctionType.Exp, scale=0.125)
                        # causal fixup on the 4 diagonal s-tiles
                        for c in range(4):
                            st = 4 * j + c
                            if c > 0:
                                nc.gpsimd.memset(at[:, st, 0 : 128 * c], 0.0)
                            csl = slice(128 * c, 128 * c + 128)
                            nc.vector.tensor_mul(at[:, st, csl], at[:, st, csl], m01_sb)
                        ct = ps2.tile([65, 512], F32, tag="ct", bufs=2)
                        for st in range(nst):
                            nc.tensor.matmul(ct, v1_sb[:, h, st, :], at[:, st, :],
                                             start=(st == 0), stop=(st == nst - 1))
                        rr = epip.tile([1, 512], F32, tag="rr")
                        nc.vector.reciprocal(rr, ct[64:65, :])
                        rb = epip.tile([64, 512], F32, tag="rb")
                        nc.gpsimd.partition_broadcast(rb, rr)
                        stg = epip.tile([64, 512], F32, tag="stg")
                        nc.vector.tensor_mul(stg, ct[0:64, :], rb)
                        nc.sync.dma_start(out=ctxT_sb[rsl, dt, qsl], in_=stg.bitcast(F32R))

            if DEBUG_DUMPS:
                nc.sync.dma_start(out=dbg_q, in_=qT_sb.bitcast(F32).rearrange("p a t -> p (a t)"))
                nc.sync.dma_start(out=dbg_k, in_=kT_sb.bitcast(F32).rearrange("p a t -> p (a t)"))
                dbg_v_sb = persist.tile([128, G * NTT * 65], F32)
                nc.vector.tensor_copy(dbg_v_sb, v1_sb.rearrange("p a b c -> p (a b c)"))
                nc.sync.dma_start(out=dbg_v, in_=dbg_v_sb)
                nc.sync.dma_start(out=dbg_c, in_=ctxT_sb.bitcast(F32).rearrange("p a t -> p (a t)"))

            # ---- phase 3: output projection (partial over this core's 256 dims) ----
            with tc.tile_pool(name="ps3", bufs=1, space="PSUM") as ps3:
                for tt in range(NTT):
                    po = ps3.tile([128, D], F32, tag="po", bufs=2)
                    for nchk in range(2):
                        for k in range(2):
                            nc.tensor.matmul(
                                po[:, 512 * nchk : 512 * nchk + 512],
                                ctxT_sb[:, k, 128 * tt : 128 * tt + 128],
                                wo_sb[:, k, 512 * nchk : 512 * nchk + 512],
                                start=(k == 0), stop=(k == 1))
                    osb = epip.tile([128, D], F32, tag="osb", bufs=3)
                    nc.vector.tensor_copy(osb, po)
                    nc.sync.dma_start(out=out[128 * tt : 128 * tt + 128, :], in_=osb)

    nc.compile()
    return nc


def _make_tables():
    i = np.arange(0, DK, 2, dtype=np.float32) / DK  # 2i/DK
    theta = 10000.0 ** i  # [32]
    pos = np.arange(T, dtype=np.float32)
    ang = pos[None, :] / theta[:, None]  # [32, T]
    sinT, cosT = np.sin(ang), np.cos(ang)
    cc = np.tile(cosT, (4, 1)).astype(np.float32)  # [128, T]
    ss = np.tile(np.concatenate([-sinT, sinT], 0), (2, 1)).astype(np.float32)
    m01 = (np.arange(128)[:, None] <= np.arange(128)[None, :])
    import ml_dtypes
    m01 = m01.astype(ml_dtypes.bfloat16)
    return cc, ss, m01


def _make_in_maps(x, wq, bq, wk, bk, wv, bv, wo):
    cc, ss, m01 = _make_tables()
    p = np.concatenate([np.arange(0, DK, 2), np.arange(1, DK, 2)])  # rope perm
    in_maps = []
    for core in range(NCORES):
        b, g = divmod(core, G)
        heads = np.arange(4 * g, 4 * g + 4)
        rows_qk = np.concatenate([64 * h + p for h in heads])
        rows_v = np.concatenate([64 * h + np.arange(DK) for h in heads])
        bqk = np.stack([bq[rows_qk[0:128]], bq[rows_qk[128:256]],
                        bk[rows_qk[0:128]], bk[rows_qk[128:256]]], axis=1)
        in_maps.append({
            "xT": np.ascontiguousarray(x[b].T),
            "wqT": np.ascontiguousarray(wq[rows_qk].T),
            "wkT": np.ascontiguousarray(wk[rows_qk].T),
            "wvT": np.ascontiguousarray(wv[rows_v].T),
            "woT": np.ascontiguousarray(wo[:, rows_v].T),
            "bqk": np.ascontiguousarray(bqk.astype(np.float32)),
            "bv": np.ascontiguousarray(bv[rows_v][None, :]),
            "cc": cc, "ss": ss, "m01": m01,
            "ones": np.ones((1, 128), np.float32),
        })
    return in_maps


def _get_runner():
    """Compile once; return a jitted 8-core runner reusable across calls."""
    if "runner" in _CACHE:
        return _CACHE["runner"]
    import jax
    from jax.sharding import Mesh, PartitionSpec
    from jax.experimental.shard_map import shard_map

    install_neuronx_cc_hook()
    nc = _build_bass()

    partition_name = nc.partition_id_tensor.name if nc.partition_id_tensor else None
    in_names, out_names, out_avals = [], [], []
    for alloc in nc.m.functions[0].allocations:
        if not isinstance(alloc, mybir.MemoryLocationSet):
            continue
        name = alloc.memorylocations[0].name
        if alloc.kind == "ExternalInput":
            if name != partition_name:
                in_names.append(name)
        elif alloc.kind == "ExternalOutput":
            out_names.append(name)
            out_avals.append(
                jax.core.ShapedArray(tuple(alloc.tensor_shape), mybir.dt.np(alloc.dtype)))
    n_params = len(in_names)
    all_in = list(in_names) + list(out_names)

    def _pid():
        import jax.numpy as jnp
        from concourse.bass2jax import partition_id_tensor
        return partition_id_tensor()

    def _body(*args):
        operands = list(args)
        if partition_name is not None:
            operands.append(_pid())
        outs = _bass_exec_p.bind(
            *operands,
            out_avals=tuple(out_avals),
            in_names=tuple(all_in + ([partition_name] if partition_name else [])),
            out_names=tuple(out_names),
            lowering_input_output_aliases=(),
            sim_require_finite=True,
            sim_require_nnan=True,
            nc=nc,
        )
        return tuple(outs)

    devices = jax.devices()[:NCORES]
    mesh = Mesh(np.asarray(devices), ("core",))
    nin = n_params + len(out_names)
    sharded = jax.jit(shard_map(
        _body, mesh=mesh,
        in_specs=(PartitionSpec("core"),) * nin,
        out_specs=(PartitionSpec("core"),) * len(out_names),
        check_rep=False))

    def run(in_maps):
        concat_in = [
            np.concatenate([np.asarray(m[nm]) for m in in_maps], axis=0)
            for nm in in_names
        ]
        zeros = [np.zeros((NCORES * a.shape[0], *a.shape[1:]), a.dtype) for a in out_avals]
        out_arrs = sharded(*concat_in, *zeros)
        o = np.asarray(out_arrs[out_names.index("out")])
        return o.reshape(NCORES, T, D)

    runner = {"run": run, "sharded": sharded, "in_names": in_names,
              "out_names": out_names, "out_avals": out_avals}
    _CACHE["runner"] = runner
    return runner


def kernel(x, wq, bq, wk, bk, wv, bv, wo, bo, attn_mask):
    x = np.asarray(x, np.float32)
    in_maps = _make_in_maps(
        x, np.asarray(wq, np.float32), np.asarray(bq, np.float32),
        np.asarray(wk, np.float32), np.asarray(bk, np.float32),
        np.asarray(wv, np.float32), np.asarray(bv, np.float32),
        np.asarray(wo, np.float32))
    parts = _get_runner()["run"](in_maps)  # [8, T, D]
    out = parts.reshape(B, G, T, D).sum(axis=1) + np.asarray(bo, np.float32)
    return out.astype(np.float32)
